# revision 3
# baseline (speedup 1.0000x reference)
"""Linear (kernelized) attention for Trainium2, data-parallel over batch N=8
across 8 NeuronCores.

Math (per batch n, head h):
  K' = elu(K)+1, Q' = elu(Q)+1          [S,D] / [L,D]
  KV = K'^T @ V                         [D,D]   (the /S and *S of the
  ksum = sum_s K'                       [D]      reference cancel exactly)
  den[l] = Q'[l,:] . ksum               [L]
  out[l,v] = (Q'[l,:] @ KV)[v] / den[l] [L,D]
eps=1e-6 in the reference is far below one ulp of den (~1e5), so 1/(den+eps)
== 1/den bitwise in fp32.

elu1(x) = exp(min(x,0)) + relu(x) is computed as
  rm = Relu(-x); e = Exp(-rm)   (two ACT ops)
  out = max(x,0) + e            (one DVE scalar_tensor_tensor)

Layout trick: 4 heads x 32 dims are packed on 128 partitions ("head group"
g in {0,1}).  Phase 1 accumulates KVfull_g = K'_g^T @ V_g (all 4x4 head
cross-blocks; only the diagonal ones are used) and ksum_g = K'_g^T @ ones
over 64 s-tiles in PSUM.  A block-diagonal rhs2_g [128(h,d), 132] =
[BD(KV) | ksum columns] is then built once.  Phase 2 transposes each elu'd
Q-tile on the PE, and a single matmul per group yields [128(l), 132] =
[numerator | denominator]; a reciprocal on [128,4] plus one broadcast
tensor-tensor multiply finish the tile.
"""

import os
from contextlib import ExitStack

import numpy as np

N, L, S, H, D = 8, 8192, 8192, 8, 32
HD = H * D  # 256
P = 128
NCORES = 8
NG = 2  # head groups of 4 heads * 32 dim = 128 partitions
GH = 4  # heads per group

_CACHE = {}


def emit_mixattention(ctx, tc, o_ap, q_ap, k_ap, v_ap, L_=L, S_=S):
    import concourse.bass as bass
    from concourse import mybir
    from concourse.masks import make_identity

    nc = tc.nc
    f32 = mybir.dt.float32
    ST = S_ // P
    LT = L_ // P

    consts = ctx.enter_context(tc.tile_pool(name="consts", bufs=1))
    io_pool = ctx.enter_context(tc.tile_pool(name="io", bufs=4))
    elw_pool = ctx.enter_context(tc.tile_pool(name="elw", bufs=4))
    qt_pool = ctx.enter_context(tc.tile_pool(name="qt", bufs=4))
    out_pool = ctx.enter_context(tc.tile_pool(name="outp", bufs=4))
    rhs2_pool = ctx.enter_context(tc.tile_pool(name="rhs2", bufs=1))
    small_pool = ctx.enter_context(tc.tile_pool(name="small", bufs=4))
    ps_acc = ctx.enter_context(tc.tile_pool(name="ps_acc", bufs=1, space="PSUM"))
    ps_t = ctx.enter_context(tc.tile_pool(name="ps_t", bufs=2, space="PSUM"))
    ps_o = ctx.enter_context(tc.tile_pool(name="ps_o", bufs=2, space="PSUM"))

    identity = consts.tile([P, P], f32)
    make_identity(nc, identity)
    ones = consts.tile([P, 1], f32)
    nc.vector.memset(ones, 1.0)

    def elu1(x_tile, tag):
        """returns tile with elu(x)+1 = max(x,0) + exp(min(x,0))"""
        rm = elw_pool.tile([P, HD], f32, tag=f"rm")
        # rm = relu(-x)
        nc.scalar.activation(out=rm, in_=x_tile, func=mybir.ActivationFunctionType.Relu,
                             scale=-1.0)
        e = elw_pool.tile([P, HD], f32, tag=f"e")
        # e = exp(-rm) = exp(min(x, 0))
        nc.scalar.activation(out=e, in_=rm, func=mybir.ActivationFunctionType.Exp,
                             scale=-1.0)
        xp = elw_pool.tile([P, HD], f32, tag=tag)
        # xp = max(x, 0) + e
        nc.vector.scalar_tensor_tensor(out=xp, in0=x_tile, scalar=0.0, in1=e,
                                       op0=mybir.AluOpType.max,
                                       op1=mybir.AluOpType.add)
        return xp

    # ---------------- Phase 1: KV + ksum accumulation over s-tiles ----------
    kv_ps = [ps_acc.tile([P, P], f32, tag=f"kv{g}", name=f"kv_ps{g}") for g in range(NG)]
    ks_ps = [ps_acc.tile([P, 1], f32, tag=f"ks{g}", name=f"ks_ps{g}") for g in range(NG)]

    for i in range(ST):
        ktile = io_pool.tile([P, HD], f32, tag="ktile")
        nc.sync.dma_start(out=ktile, in_=k_ap[i * P:(i + 1) * P, :])
        vtile = io_pool.tile([P, HD], f32, tag="vtile")
        nc.sync.dma_start(out=vtile, in_=v_ap[i * P:(i + 1) * P, :])
        kp = elu1(ktile, "kp")
        first, last = (i == 0), (i == ST - 1)
        for g in range(NG):
            lhsT = kp[:, g * P:(g + 1) * P]
            nc.tensor.matmul(kv_ps[g], lhsT, vtile[:, g * P:(g + 1) * P],
                             start=first, stop=last)
            nc.tensor.matmul(ks_ps[g], lhsT, ones, start=first, stop=last)

    # ---------------- build rhs2_g = [BD(KV_h) | ksum cols] [128, 132] ------
    rhs2 = []
    for g in range(NG):
        r2 = rhs2_pool.tile([P, 132], f32, tag=f"rhs2_{g}", name=f"rhs2_{g}")
        nc.vector.memset(r2, 0.0)
        for h in range(GH):
            sl = slice(h * D, (h + 1) * D)
            nc.scalar.copy(out=r2[sl, sl], in_=kv_ps[g][sl, sl])
            nc.scalar.copy(out=r2[sl, P + h:P + h + 1], in_=ks_ps[g][sl, 0:1])
        rhs2.append(r2)

    # ---------------- Phase 2: per l-tile -----------------------------------
    for j in range(LT):
        qtile = io_pool.tile([P, HD], f32, tag="qtile")
        nc.sync.dma_start(out=qtile, in_=q_ap[j * P:(j + 1) * P, :])
        qp = elu1(qtile, "qp")
        ot = out_pool.tile([P, HD], f32, tag="ot")
        for g in range(NG):
            tp = ps_t.tile([P, P], f32, tag="tp")
            nc.tensor.transpose(tp, qp[:, g * P:(g + 1) * P], identity)
            qt = qt_pool.tile([P, P], f32, tag="qt")
            nc.scalar.copy(out=qt, in_=tp)
            po = ps_o.tile([P, 132], f32, tag="po")
            nc.tensor.matmul(po, qt, rhs2[g], start=True, stop=True)
            rden = small_pool.tile([P, GH], f32, tag="rden")
            nc.vector.reciprocal(rden, po[:, P:P + GH])
            # ot[:, g*128 + (h,v)] = po[:, (h,v)] * rden[:, h]
            num = po[:, 0:P].rearrange("p (h v) -> p h v", h=GH)
            dst = ot[:, g * P:(g + 1) * P].rearrange("p (h v) -> p h v", h=GH)
            rb = rden[:, :].unsqueeze(2).broadcast_to((P, GH, D))
            nc.vector.tensor_mul(out=dst, in0=num, in1=rb)
        nc.sync.dma_start(out=o_ap[j * P:(j + 1) * P, :], in_=ot)


def _build(L_=L, S_=S):
    import concourse.bacc as bacc
    import concourse.tile as tile
    from concourse import mybir

    nc = bacc.Bacc("TRN2", target_bir_lowering=False, debug=False,
                   num_devices=NCORES)
    f32 = mybir.dt.float32
    q = nc.dram_tensor("q", [L_, HD], f32, kind="ExternalInput").ap()
    k = nc.dram_tensor("k", [S_, HD], f32, kind="ExternalInput").ap()
    v = nc.dram_tensor("v", [S_, HD], f32, kind="ExternalInput").ap()
    o = nc.dram_tensor("o", [L_, HD], f32, kind="ExternalOutput").ap()
    with tile.TileContext(nc) as tc:
        with ExitStack() as ctx:
            emit_mixattention(ctx, tc, o, q, k, v, L_, S_)
    nc.compile()
    return nc


def kernel(queries, keys, values):
    from concourse.bass_utils import run_bass_kernel_spmd

    if "nc" not in _CACHE:
        _CACHE["nc"] = _build()
    nc = _CACHE["nc"]

    in_maps = []
    for i in range(NCORES):
        in_maps.append({
            "q": np.ascontiguousarray(np.asarray(queries[i], np.float32).reshape(L, HD)),
            "k": np.ascontiguousarray(np.asarray(keys[i], np.float32).reshape(S, HD)),
            "v": np.ascontiguousarray(np.asarray(values[i], np.float32).reshape(S, HD)),
        })
    res = run_bass_kernel_spmd(nc, in_maps, core_ids=list(range(NCORES)),
                               trace=os.environ.get("BASS_KERNEL_TRACE", "0") == "1")
    _CACHE["last_result"] = res
    out = np.stack([res.results[i]["o"].reshape(L, H, D) for i in range(NCORES)])
    return out


# revision 15
# speedup vs baseline: 264.6404x; 264.6404x over previous
"""Linear (kernelized) attention for Trainium2, data-parallel over batch N=8
across 8 NeuronCores.

Math (per batch n, head h):
  K' = elu(K)+1, Q' = elu(Q)+1          [S,D] / [L,D]
  KV = K'^T @ V                         [D,D]   (the /S and *S of the
  ksum = sum_s K'                       [D]      reference cancel exactly)
  den[l] = Q'[l,:] . ksum               [L]
  out[l,v] = (Q'[l,:] @ KV)[v] / den[l] [L,D]
eps=1e-6 in the reference is far below one ulp of den (~1e5), so 1/(den+eps)
== 1/den bitwise in fp32.

elu1(x) = exp(min(x,0)) + relu(x) is computed as
  rm = Relu(-x); e = Exp(-rm)   (two ACT ops)
  out = max(x,0) + e            (one DVE scalar_tensor_tensor)

Layout: 4 heads x 32 dims are packed on 128 partitions ("head group"
g in {0,1}).  Phase 1 accumulates KVfull_g = K'_g^T @ V_g (all 4x4 head
cross-blocks; only the diagonal ones are used) and ksum_g = K'_g^T @ ones
over the 64 s-tiles in PSUM.  A block-diagonal rhs2_g [128(h,d), 132] =
[BD(KV) | ksum columns] is then built once.  Phase 2 transposes each elu'd
Q-tile on the PE, and a single matmul per group yields [128(l), 132] =
[numerator | denominator]; a reciprocal plus one broadcast multiply finish.

Perf structure: all HBM traffic moves in supertiles of TS=4 row-tiles per
dma_start (HWDGE fixed cost is ~625ns/DMA), elementwise ops run on whole
[128, 4*256] supertiles, and the two per-subtile PE transposes share one
PSUM tile so a single [128,256] copy materializes both lhsT halves.
"""

import os
from contextlib import ExitStack

import numpy as np

N, L, S, H, D = 8, 8192, 8192, 8, 32
HD = H * D  # 256
P = 128
NCORES = 8
NG = 2  # head groups of 4 heads * 32 dim = 128 partitions
GH = 4  # heads per group
TS = int(os.environ.get("KTS", "8"))  # row-tiles per supertile / DMA
KB = int(os.environ.get("KBUF", "2"))  # default buffer depth
QTB = int(os.environ.get("KQTB", "8"))  # qt tile bufs
HF = 4  # phase-2 po half-supertile (PSUM bank budget)

_CACHE = {}
QB = int(os.environ.get("KQB", "2"))


def emit_mixattention(ctx, tc, o_ap, q_ap, k_ap, v_ap, L_=L, S_=S, repeat=1, phases="12"):
    from concourse import mybir
    from concourse.masks import make_identity

    nc = tc.nc
    f32 = mybir.dt.float32

    consts = ctx.enter_context(tc.tile_pool(name="consts", bufs=1))
    io_pool = ctx.enter_context(tc.tile_pool(name="io", bufs=3))
    elw_pool = ctx.enter_context(tc.tile_pool(name="elw", bufs=2))
    qt_pool = ctx.enter_context(tc.tile_pool(name="qt", bufs=6))
    out_pool = ctx.enter_context(tc.tile_pool(name="outp", bufs=3))
    rhs2_pool = ctx.enter_context(tc.tile_pool(name="rhs2", bufs=1))
    small_pool = ctx.enter_context(tc.tile_pool(name="small", bufs=4))
    ps_acc = ctx.enter_context(tc.tile_pool(name="ps_acc", bufs=1, space="PSUM"))
    ps_t = ctx.enter_context(tc.tile_pool(name="ps_t", bufs=2, space="PSUM"))
    ps_o = ctx.enter_context(tc.tile_pool(name="ps_o", bufs=2, space="PSUM"))

    identity = consts.tile([P, P], f32)
    make_identity(nc, identity)
    ones = consts.tile([P, 1], f32)
    nc.vector.memset(ones, 1.0)

    pools = (io_pool, elw_pool, qt_pool, out_pool, rhs2_pool, small_pool,
             ps_acc, ps_t, ps_o)

    def _body():
        _emit_body(tc, o_ap, q_ap, k_ap, v_ap, L_, S_, identity, ones, phases, *pools)

    if repeat == 1:
        _body()
    else:
        with tc.For_i(0, repeat, 1):
            _body()


def _emit_body(tc, o_ap, q_ap, k_ap, v_ap, L_, S_, identity, ones, phases,
               io_pool, elw_pool, qt_pool, out_pool, rhs2_pool, small_pool,
               ps_acc, ps_t, ps_o):
    from concourse import mybir

    nc = tc.nc
    f32 = mybir.dt.float32
    ts = min(TS, S_ // P, L_ // P)  # subtiles per supertile
    hf = min(HF, ts)
    SROWS = ts * P  # rows per supertile
    NST = S_ // SROWS  # number of K/V supertiles
    NLT = L_ // SROWS  # number of Q/O supertiles

    def super_ap(dram, t):
        """[128, ts, HD] view of DRAM rows t*SROWS..(t+1)*SROWS"""
        return dram[t * SROWS:(t + 1) * SROWS, :].rearrange(
            "(c p) d -> p c d", p=P)

    def elu1(x_tile, tag, bufs=None):
        """elu(x)+1 = max(x,0) + exp(min(x,0)) on a [P,ts,HD] supertile"""
        m = elw_pool.tile([P, ts, HD], f32, tag="m", name="m", bufs=KB)
        nc.vector.tensor_scalar_min(m, x_tile, 0.0)
        e = elw_pool.tile([P, ts, HD], f32, tag="e", name="e", bufs=KB)
        nc.scalar.activation(out=e, in_=m,
                             func=mybir.ActivationFunctionType.Exp, scale=1.0)
        xp = elw_pool.tile([P, ts, HD], f32, tag=tag, name=tag, bufs=bufs or KB)
        nc.vector.scalar_tensor_tensor(out=xp, in0=x_tile, scalar=0.0, in1=e,
                                       op0=mybir.AluOpType.max,
                                       op1=mybir.AluOpType.add)
        return xp

    # ---------------- Phase 1: KV + ksum accumulation -----------------------
    # vtile subtile layout [V_g0 | 1 | V_g1 | 1] (258 cols) so that
    # rhs_g = vtile[:, c, g*129:(g+1)*129] = [V_g | ones] is contiguous and a
    # single matmul per (c, g) accumulates both KV (cols 0:128) and ksum
    # (col 128) into acc_g [128, 129].
    VW = P + 1  # 129
    acc = [ps_acc.tile([P, NG * VW], f32, tag=f"acc{g}", name=f"acc{g}")
           for g in range(NG)]

    if "1" not in phases:
        for g in range(NG):
            nc.vector.memset(acc[g], 1.0)
    for i in range(NST if "1" in phases else 0):
        ktile = io_pool.tile([P, ts, HD], f32, tag="ktile", name="ktile", bufs=KB)
        nc.gpsimd.dma_start(out=ktile, in_=super_ap(k_ap, i))
        vtile = io_pool.tile([P, ts, 2 * VW], f32, tag="vtile", name="vtile", bufs=KB)
        nc.gpsimd.memset(vtile[:, :, P:P + 1], 1.0)
        nc.gpsimd.memset(vtile[:, :, P + VW:P + VW + 1], 1.0)
        for g in range(NG):
            nc.sync.dma_start(
                out=vtile[:, :, g * VW:g * VW + P],
                in_=super_ap(v_ap, i)[:, :, g * P:(g + 1) * P])
        kp = elu1(ktile, "kp")
        for c in range(ts):
            first = (i == 0 and c == 0)
            last = (i == NST - 1 and c == ts - 1)
            for g in range(NG):
                nc.tensor.matmul(acc[g][:, 0:VW], kp[:, c, g * P:(g + 1) * P],
                                 vtile[:, c, g * VW:(g + 1) * VW],
                                 start=first, stop=last)

    # ---------------- build rhs2_g = [BD(KV_h) | ksum cols] [128, 132] ------
    rhs2 = []
    for g in range(NG):
        r2 = rhs2_pool.tile([P, 132], f32, tag=f"rhs2_{g}", name=f"rhs2_{g}")
        nc.vector.memset(r2, 0.0)
        for h in range(GH):
            sl = slice(h * D, (h + 1) * D)
            nc.scalar.copy(out=r2[sl, sl], in_=acc[g][sl, sl])
            nc.scalar.copy(out=r2[sl, P + h:P + h + 1], in_=acc[g][sl, P:P + 1])
        rhs2.append(r2)

    # ---------------- Phase 2: per q/o supertile ----------------------------
    if "2" not in phases:
        nc.sync.dma_start(out=o_ap[0:P, 0:132], in_=rhs2[0])
    for j in range(NLT if "2" in phases else 0):
        qtile = io_pool.tile([P, ts, HD], f32, tag="qtile", name="qtile", bufs=QB)
        nc.gpsimd.dma_start(out=qtile, in_=super_ap(q_ap, j))
        qp = elu1(qtile, "qp", bufs=KB)
        # transpose both groups of each subtile into one PSUM tile, then one
        # copy -> qt[c] [128, 2, 128] = lhsT for both groups
        qts = []
        for c in range(ts):
            tp = ps_t.tile([P, NG, P], f32, tag="tp", name="tp")
            for g in range(NG):
                # both transposes share one PSUM bank: only the first may
                # carry start=True (start zeroes the whole 2KB zero-region)
                nc.tensor.matmul(tp[:, g, :], qp[:, c, g * P:(g + 1) * P],
                                 identity, is_transpose=True,
                                 start=(g == 0), stop=(g == NG - 1))
            qt = qt_pool.tile([P, NG, P], f32, tag="qt", name="qt", bufs=QTB)
            nc.scalar.copy(out=qt, in_=tp)
            qts.append(qt)
        ot = out_pool.tile([P, ts, HD], f32, tag="ot", name="ot", bufs=KB)
        for g in range(NG):
            for hb in range(0, ts, hf):
                # [128, hf, 256]: per subtile 1KB -> no PSUM bank straddle
                po = ps_o.tile([P, hf, HD], f32, tag="po", name="po")
                for ci in range(hf):
                    c = hb + ci
                    # subtiles ci, ci+1 share a PSUM bank: start on even ci
                    nc.tensor.matmul(po[:, ci, 0:132], qts[c][:, g, :],
                                     rhs2[g],
                                     start=(ci % 2 == 0), stop=(ci % 2 == 1))
                rden = small_pool.tile([P, hf, GH], f32, tag="rden",
                                       name="rden")
                nc.vector.reciprocal(rden, po[:, :, P:P + GH])
                num = po[:, :, 0:P].rearrange("p c (h v) -> p c h v", h=GH)
                dst = ot[:, hb:hb + hf, g * P:(g + 1) * P].rearrange(
                    "p c (h v) -> p c h v", h=GH)
                rb = rden[:, :, :].unsqueeze(3).broadcast_to((P, hf, GH, D))
                nc.vector.tensor_mul(out=dst, in0=num, in1=rb)
        nc.sync.dma_start(out=super_ap(o_ap, j), in_=ot)


def _build(L_=L, S_=S, repeat=1, phases="12"):
    import concourse.bacc as bacc
    import concourse.tile as tile
    from concourse import mybir

    nc = bacc.Bacc("TRN2", target_bir_lowering=False, debug=False,
                   num_devices=NCORES)
    f32 = mybir.dt.float32
    q = nc.dram_tensor("q", [L_, HD], f32, kind="ExternalInput").ap()
    k = nc.dram_tensor("k", [S_, HD], f32, kind="ExternalInput").ap()
    v = nc.dram_tensor("v", [S_, HD], f32, kind="ExternalInput").ap()
    o = nc.dram_tensor("o", [L_, HD], f32, kind="ExternalOutput").ap()
    with tile.TileContext(nc) as tc:
        with ExitStack() as ctx:
            emit_mixattention(ctx, tc, o, q, k, v, L_, S_, repeat=repeat, phases=phases)
    nc.compile()
    return nc


def kernel(queries, keys, values):
    from concourse.bass_utils import run_bass_kernel_spmd

    if "nc" not in _CACHE:
        _CACHE["nc"] = _build()
    nc = _CACHE["nc"]

    in_maps = []
    for i in range(NCORES):
        in_maps.append({
            "q": np.ascontiguousarray(np.asarray(queries[i], np.float32).reshape(L, HD)),
            "k": np.ascontiguousarray(np.asarray(keys[i], np.float32).reshape(S, HD)),
            "v": np.ascontiguousarray(np.asarray(values[i], np.float32).reshape(S, HD)),
        })
    res = run_bass_kernel_spmd(nc, in_maps, core_ids=list(range(NCORES)),
                               trace=os.environ.get("BASS_KERNEL_TRACE", "0") == "1")
    _CACHE["last_result"] = res
    out = np.stack([res.results[i]["o"].reshape(L, H, D) for i in range(NCORES)])
    return out
